# revision 4
# baseline (speedup 1.0000x reference)
"""Trilinear interpolation (trilerp) kernel for trn2, 8 NeuronCores.

Strategy:
 - Host shards the 4M points across 8 cores by x-slab bucket (i0x >> 4), a
   locality-aware data-parallel sharding of the point dimension.
 - Host replicates the (small) grid table to every core in a rearranged
   "cell-corner" layout R64: for each cell (i_rel, j, k) a contiguous 64B
   block holding all 8 corner values x 2 features, so the device gathers
   exactly one 64B block per point.
 - Device (per core): computes grid coords, fractional weights and flat cell
   indices on ACT/DVE, gathers the 64B blocks via GPSIMD indirect DMA
   (128 points per instruction, one offset per partition), and blends with
   the trilinear weights on DVE.
 - Host inverse-permutes the per-core outputs back to the original order.

All trilerp arithmetic (indices, weights, gather, blend) happens on device;
the host only permutes/lays out data.
"""
import sys
sys.path.insert(0, '/opt/trn_rl_repo')
import numpy as np

import concourse.bass as bass
import concourse.mybir as mybir
from concourse import bacc
from concourse.tile import TileContext
from concourse.bass_utils import run_bass_kernel_spmd

RES = 128
F = 2
NCORES = 8
P = 128          # partitions
CH = 128         # columns per chunk (=> 128 gathers of 128 points per chunk)
ROWS = 16 * RES * RES   # R64 rows per core (16 x-slabs x 128 x 128 cells)

_F32 = mybir.dt.float32
_I32 = mybir.dt.int32
_AF = mybir.ActivationFunctionType
_OP = mybir.AluOpType


def build_core_kernel(n_cols, rows=ROWS, chunk=CH):
    """One-core bass program: [128, n_cols] points, gather table R [rows, 16]."""
    assert n_cols % chunk == 0
    nc = bacc.Bacc("TRN2", target_bir_lowering=False, debug=False,
                   num_devices=NCORES)
    c0 = nc.dram_tensor("c0", [P, n_cols], _F32, kind="ExternalInput")
    c1 = nc.dram_tensor("c1", [P, n_cols], _F32, kind="ExternalInput")
    c2 = nc.dram_tensor("c2", [P, n_cols], _F32, kind="ExternalInput")
    R = nc.dram_tensor("R", [rows, 16], _F32, kind="ExternalInput")
    biasm = nc.dram_tensor("biasm", [P, 1], _F32, kind="ExternalInput")
    out = nc.dram_tensor("out", [P, n_cols, F], _F32, kind="ExternalOutput")

    nchunks = n_cols // chunk
    with TileContext(nc) as tc:
        with tc.tile_pool(name="io", bufs=1) as io, \
             tc.tile_pool(name="wk", bufs=2) as wk, \
             tc.tile_pool(name="gp", bufs=2) as gp:
            bias_sb = io.tile([P, 1], _F32)
            nc.sync.dma_start(out=bias_sb[:], in_=biasm[:])
            for q in range(nchunks):
                sl = slice(q * chunk, (q + 1) * chunk)
                t0 = wk.tile([P, chunk], _F32, tag="t0")
                t1 = wk.tile([P, chunk], _F32, tag="t1")
                t2 = wk.tile([P, chunk], _F32, tag="t2")
                nc.sync.dma_start(out=t0[:], in_=c0[:, sl])
                nc.sync.dma_start(out=t1[:], in_=c1[:, sl])
                nc.sync.dma_start(out=t2[:], in_=c2[:, sl])

                # xs = 127*c  (ACT, Copy w/ scale; float32, same rounding as host)
                xs0 = wk.tile([P, chunk], _F32, tag="xs0")
                xs1 = wk.tile([P, chunk], _F32, tag="xs1")
                xs2 = wk.tile([P, chunk], _F32, tag="xs2")
                nc.scalar.mul(xs0[:], t0[:], float(RES - 1))
                nc.scalar.mul(xs1[:], t1[:], float(RES - 1))
                nc.scalar.mul(xs2[:], t2[:], float(RES - 1))

                # floor via round-to-nearest (+2^23 - 2^23) then correct:
                #   t = rn(xs); gt = (t > xs); i0 = t - gt; fr = xs - i0
                # exact for 0 <= xs < 2^22.
                C23 = float(1 << 23)
                wz = wk.tile([P, chunk, 2], _F32, tag="wz")
                fr0 = wk.tile([P, chunk], _F32, tag="fr0")
                fr1 = wk.tile([P, chunk], _F32, tag="fr1")
                i0x = wk.tile([P, chunk], _F32, tag="i0x")
                i0y = wk.tile([P, chunk], _F32, tag="i0y")
                i0z = wk.tile([P, chunk], _F32, tag="i0z")
                tt = wk.tile([P, chunk], _F32, tag="tt")
                gt = wk.tile([P, chunk], _F32, tag="gt")
                for xs_d, i0_d, fr_ap in (
                    (xs0, i0x, fr0[:]),
                    (xs1, i0y, fr1[:]),
                    (xs2, i0z, wz[:, :, 1]),
                ):
                    nc.vector.tensor_scalar(tt[:], xs_d[:], C23, C23,
                                            _OP.add, _OP.subtract)
                    nc.vector.tensor_tensor(gt[:], tt[:], xs_d[:], _OP.is_gt)
                    nc.vector.tensor_tensor(i0_d[:], tt[:], gt[:], _OP.subtract)
                    nc.vector.tensor_tensor(fr_ap, xs_d[:], i0_d[:], _OP.subtract)

                # m = 16384*i0x + 128*i0y + i0z + bias (bias = -16c*16384)
                ax = wk.tile([P, chunk], _F32, tag="ax")
                by = wk.tile([P, chunk], _F32, tag="by")
                nc.scalar.activation(ax[:], i0x[:], _AF.Identity,
                                     bias=bias_sb[:, 0:1], scale=float(16384))
                nc.scalar.mul(by[:], i0y[:], float(128))
                mf = wk.tile([P, chunk], _F32, tag="mf")
                nc.vector.tensor_tensor(mf[:], ax[:], by[:], _OP.add)
                nc.vector.tensor_tensor(mf[:], mf[:], i0z[:], _OP.add)
                mi = wk.tile([P, chunk], _I32, tag="mi")
                nc.vector.tensor_copy(out=mi[:], in_=mf[:])

                # weights
                omx = wk.tile([P, chunk], _F32, tag="omx")
                omy = wk.tile([P, chunk], _F32, tag="omy")
                nc.scalar.activation(omx[:], fr0[:], _AF.Identity, bias=1.0, scale=-1.0)
                nc.scalar.activation(omy[:], fr1[:], _AF.Identity, bias=1.0, scale=-1.0)
                nc.scalar.activation(wz[:, :, 0], wz[:, :, 1], _AF.Identity,
                                     bias=1.0, scale=-1.0)
                wxy = wk.tile([P, chunk, 4], _F32, tag="wxy")
                nc.vector.tensor_tensor(wxy[:, :, 0], omx[:], omy[:], _OP.mult)
                nc.vector.tensor_tensor(wxy[:, :, 1], omx[:], fr1[:], _OP.mult)
                nc.vector.tensor_tensor(wxy[:, :, 2], fr0[:], omy[:], _OP.mult)
                nc.vector.tensor_tensor(wxy[:, :, 3], fr0[:], fr1[:], _OP.mult)
                # w8[p, t, c4, z] = wxy[p, t, c4] * wz[p, t, z]
                w8 = wk.tile([P, chunk, 4, 2], _F32, tag="w8")
                nc.vector.tensor_tensor(
                    w8[:],
                    wxy[:].unsqueeze(-1).broadcast_to([P, chunk, 4, 2]),
                    wz[:].unsqueeze(2).broadcast_to([P, chunk, 4, 2]),
                    _OP.mult)

                # gather: one 64B block per point, 128 points per instruction
                g = gp.tile([P, chunk, 16], _F32, tag="g")
                for t in range(chunk):
                    nc.gpsimd.indirect_dma_start(
                        out=g[:, t, :], out_offset=None, in_=R[:],
                        in_offset=bass.IndirectOffsetOnAxis(
                            ap=mi[:, t:t + 1], axis=0))

                # blend: P8 = G * W8 (broadcast over f), then sum over (c4, z)
                p8 = gp.tile([P, chunk, 8, F], _F32, tag="p8")
                gv = g[:].rearrange("p t (a f) -> p t a f", a=8, f=F)
                nc.vector.tensor_tensor(
                    p8[:], gv,
                    w8[:].rearrange("p t a z -> p t (a z)")
                        .unsqueeze(-1).broadcast_to([P, chunk, 8, F]),
                    _OP.mult)
                oc = wk.tile([P, chunk, F], _F32, tag="oc")
                nc.vector.tensor_reduce(
                    oc[:], p8[:].transpose([0, 1, 3, 2]),
                    axis=mybir.AxisListType.X, op=_OP.add)
                nc.sync.dma_start(out=out[:, sl, :], in_=oc[:])
    nc.compile()
    return nc


def _build_r64(table, x0):
    """R64 for x-slabs [x0, x0+16): [16*128*128, 16] f32.

    R64[(i,j,k)] = concat over (dx,dy) of table[x0+i+dx, j+dy, k:k+2, :]
    with edge clamping (out-of-range corners are never gathered).
    """
    T = np.ascontiguousarray(table, dtype=np.float32)
    xi = np.minimum(x0 + np.arange(16), RES - 1)
    out = np.empty((16, RES, RES, 4, 2, F), np.float32)
    k0 = np.arange(RES)
    k1 = np.minimum(k0 + 1, RES - 1)
    for dx in (0, 1):
        xs = np.minimum(xi + dx, RES - 1)
        for dy in (0, 1):
            ys = np.minimum(np.arange(RES) + dy, RES - 1)
            A = T[xs][:, ys]                       # [16, 128, 128, F]
            out[:, :, :, dx * 2 + dy, 0, :] = A[:, :, k0, :]
            out[:, :, :, dx * 2 + dy, 1, :] = A[:, :, k1, :]
    return out.reshape(ROWS, 16)


def _shard(c0):
    xs = c0 * np.float32(RES - 1)
    i0x_host = np.clip(np.floor(xs).astype(np.int64), 0, RES - 2)
    buckets = (i0x_host >> 4).astype(np.int64)
    perm = np.argsort(buckets, kind="stable")
    counts = np.bincount(buckets, minlength=NCORES)
    starts = np.concatenate([[0], np.cumsum(counts)[:-1]])
    maxc = int(counts.max())
    n_cols = max(1, -(-maxc // (P * CH))) * CH
    return perm, counts, starts, n_cols


def prepare_in_maps(c0, c1, c2, table):
    c0 = np.asarray(c0, np.float32)
    c1 = np.asarray(c1, np.float32)
    c2 = np.asarray(c2, np.float32)
    table = np.asarray(table, np.float32)
    perm, counts, starts, n_cols = _shard(c0)
    in_maps = []
    for c in range(NCORES):
        idx_c = perm[starts[c]:starts[c] + counts[c]]
        npad = P * n_cols
        # pad coords must land inside this core's x-slab bucket
        a0 = np.full(npad, np.float32((16 * c + 8.5) / (RES - 1)), np.float32)
        a1 = np.full(npad, np.float32(0.5), np.float32)
        a2 = np.full(npad, np.float32(0.5), np.float32)
        a0[:counts[c]] = c0[idx_c]
        a1[:counts[c]] = c1[idx_c]
        a2[:counts[c]] = c2[idx_c]
        in_maps.append({
            "c0": a0.reshape(P, n_cols),
            "c1": a1.reshape(P, n_cols),
            "c2": a2.reshape(P, n_cols),
            "R": _build_r64(table, 16 * c),
            "biasm": np.full((P, 1), -np.float32(16 * c * 16384), np.float32),
        })
    return in_maps, n_cols


def kernel(c0, c1, c2, table):
    c0 = np.asarray(c0, np.float32)
    N = c0.shape[0]
    perm, counts, starts, n_cols = _shard(c0)
    in_maps, _ = prepare_in_maps(c0, c1, c2, table)
    nc = build_core_kernel(n_cols)
    res = run_bass_kernel_spmd(nc, in_maps, core_ids=list(range(NCORES)))
    out_full = np.empty((N, F), np.float32)
    for c in range(NCORES):
        idx_c = perm[starts[c]:starts[c] + counts[c]]
        oc = np.asarray(res.results[c]["out"]).reshape(P * n_cols, F)
        out_full[idx_c] = oc[:counts[c]]
    return out_full


# revision 5
# speedup vs baseline: 3.8824x; 3.8824x over previous
"""Trilinear interpolation (trilerp) kernel for trn2, 8 NeuronCores.

Strategy:
 - Host shards the 4M points across 8 cores by x-slab bucket (i0x >> 4), a
   locality-aware data-parallel sharding of the point dimension.
 - Host replicates the (small) grid table to every core in a rearranged
   "cell-corner" layout R64: for each cell (i_rel, j, k) a contiguous 64B
   block holding all 8 corner values x 2 features, so the device gathers
   exactly one 64B block per point.
 - Device (per core): computes grid coords, fractional weights and flat cell
   indices on ACT/DVE, gathers the 64B blocks via GPSIMD indirect DMA
   (128 points per instruction, one offset per partition), and blends with
   the trilinear weights on DVE.
 - Host inverse-permutes the per-core outputs back to the original order.

All trilerp arithmetic (indices, weights, gather, blend) happens on device;
the host only permutes/lays out data.
"""
import sys
sys.path.insert(0, '/opt/trn_rl_repo')
import numpy as np

import concourse.bass as bass
import concourse.mybir as mybir
from concourse import bacc
from concourse.tile import TileContext
from concourse.bass_utils import run_bass_kernel_spmd

RES = 128
F = 2
NCORES = 8
P = 128          # partitions
CH = 128         # columns per chunk (=> 128 gathers of 128 points per chunk)
ROWS = 16 * RES * RES   # R64 rows per core (16 x-slabs x 128 x 128 cells)

_F32 = mybir.dt.float32
_I32 = mybir.dt.int32
_AF = mybir.ActivationFunctionType
_OP = mybir.AluOpType


def build_core_kernel(n_cols, rows=ROWS, chunk=CH):
    """One-core bass program: [128, n_cols] points, gather table R [rows, 16]."""
    assert n_cols % chunk == 0
    nc = bacc.Bacc("TRN2", target_bir_lowering=False, debug=False,
                   num_devices=NCORES, num_swdge_queues=4)
    c0 = nc.dram_tensor("c0", [P, n_cols], _F32, kind="ExternalInput")
    c1 = nc.dram_tensor("c1", [P, n_cols], _F32, kind="ExternalInput")
    c2 = nc.dram_tensor("c2", [P, n_cols], _F32, kind="ExternalInput")
    R = nc.dram_tensor("R", [rows, 16], _F32, kind="ExternalInput")
    biasm = nc.dram_tensor("biasm", [P, 1], _F32, kind="ExternalInput")
    out = nc.dram_tensor("out", [P, n_cols, F], _F32, kind="ExternalOutput")

    nchunks = n_cols // chunk
    with TileContext(nc) as tc:
        with tc.tile_pool(name="io", bufs=1) as io, \
             tc.tile_pool(name="wk", bufs=2) as wk, \
             tc.tile_pool(name="gp", bufs=2) as gp:
            bias_sb = io.tile([P, 1], _F32)
            nc.sync.dma_start(out=bias_sb[:], in_=biasm[:])
            for q in range(nchunks):
                sl = slice(q * chunk, (q + 1) * chunk)
                t0 = wk.tile([P, chunk], _F32, tag="t0")
                t1 = wk.tile([P, chunk], _F32, tag="t1")
                t2 = wk.tile([P, chunk], _F32, tag="t2")
                nc.sync.dma_start(out=t0[:], in_=c0[:, sl])
                nc.sync.dma_start(out=t1[:], in_=c1[:, sl])
                nc.sync.dma_start(out=t2[:], in_=c2[:, sl])

                # xs = 127*c  (ACT, Copy w/ scale; float32, same rounding as host)
                xs0 = wk.tile([P, chunk], _F32, tag="xs0")
                xs1 = wk.tile([P, chunk], _F32, tag="xs1")
                xs2 = wk.tile([P, chunk], _F32, tag="xs2")
                nc.scalar.mul(xs0[:], t0[:], float(RES - 1))
                nc.scalar.mul(xs1[:], t1[:], float(RES - 1))
                nc.scalar.mul(xs2[:], t2[:], float(RES - 1))

                # floor via round-to-nearest (+2^23 - 2^23) then correct:
                #   t = rn(xs); gt = (t > xs); i0 = t - gt; fr = xs - i0
                # exact for 0 <= xs < 2^22.
                C23 = float(1 << 23)
                wz = wk.tile([P, chunk, 2], _F32, tag="wz")
                fr0 = wk.tile([P, chunk], _F32, tag="fr0")
                fr1 = wk.tile([P, chunk], _F32, tag="fr1")
                i0x = wk.tile([P, chunk], _F32, tag="i0x")
                i0y = wk.tile([P, chunk], _F32, tag="i0y")
                i0z = wk.tile([P, chunk], _F32, tag="i0z")
                tt = wk.tile([P, chunk], _F32, tag="tt")
                gt = wk.tile([P, chunk], _F32, tag="gt")
                for xs_d, i0_d, fr_ap in (
                    (xs0, i0x, fr0[:]),
                    (xs1, i0y, fr1[:]),
                    (xs2, i0z, wz[:, :, 1]),
                ):
                    nc.vector.tensor_scalar(tt[:], xs_d[:], C23, C23,
                                            _OP.add, _OP.subtract)
                    nc.vector.tensor_tensor(gt[:], tt[:], xs_d[:], _OP.is_gt)
                    nc.vector.tensor_tensor(i0_d[:], tt[:], gt[:], _OP.subtract)
                    nc.vector.tensor_tensor(fr_ap, xs_d[:], i0_d[:], _OP.subtract)

                # m = 16384*i0x + 128*i0y + i0z + bias (bias = -16c*16384)
                ax = wk.tile([P, chunk], _F32, tag="ax")
                by = wk.tile([P, chunk], _F32, tag="by")
                nc.scalar.activation(ax[:], i0x[:], _AF.Identity,
                                     bias=bias_sb[:, 0:1], scale=float(16384))
                nc.scalar.mul(by[:], i0y[:], float(128))
                mf = wk.tile([P, chunk], _F32, tag="mf")
                nc.vector.tensor_tensor(mf[:], ax[:], by[:], _OP.add)
                nc.vector.tensor_tensor(mf[:], mf[:], i0z[:], _OP.add)
                mi = wk.tile([P, chunk], _I32, tag="mi")
                nc.vector.tensor_copy(out=mi[:], in_=mf[:])

                # weights
                omx = wk.tile([P, chunk], _F32, tag="omx")
                omy = wk.tile([P, chunk], _F32, tag="omy")
                nc.scalar.activation(omx[:], fr0[:], _AF.Identity, bias=1.0, scale=-1.0)
                nc.scalar.activation(omy[:], fr1[:], _AF.Identity, bias=1.0, scale=-1.0)
                nc.scalar.activation(wz[:, :, 0], wz[:, :, 1], _AF.Identity,
                                     bias=1.0, scale=-1.0)
                wxy = wk.tile([P, chunk, 4], _F32, tag="wxy")
                nc.vector.tensor_tensor(wxy[:, :, 0], omx[:], omy[:], _OP.mult)
                nc.vector.tensor_tensor(wxy[:, :, 1], omx[:], fr1[:], _OP.mult)
                nc.vector.tensor_tensor(wxy[:, :, 2], fr0[:], omy[:], _OP.mult)
                nc.vector.tensor_tensor(wxy[:, :, 3], fr0[:], fr1[:], _OP.mult)
                # w8[p, t, c4, z] = wxy[p, t, c4] * wz[p, t, z]
                w8 = wk.tile([P, chunk, 4, 2], _F32, tag="w8")
                nc.vector.tensor_tensor(
                    w8[:],
                    wxy[:].unsqueeze(-1).broadcast_to([P, chunk, 4, 2]),
                    wz[:].unsqueeze(2).broadcast_to([P, chunk, 4, 2]),
                    _OP.mult)

                # gather: one 64B block per point, 128 points per instruction
                g = gp.tile([P, chunk, 16], _F32, tag="g")
                for t in range(chunk):
                    inst = nc.gpsimd.indirect_dma_start(
                        out=g[:, t, :], out_offset=None, in_=R[:],
                        in_offset=bass.IndirectOffsetOnAxis(
                            ap=mi[:, t:t + 1], axis=0))
                    qi = t % 4
                    if qi:
                        inst.ins.queue = f"qPoolDynamic{qi}"


                # blend: P8 = G * W8 (broadcast over f), then sum over (c4, z)
                p8 = gp.tile([P, chunk, 8, F], _F32, tag="p8")
                gv = g[:].rearrange("p t (a f) -> p t a f", a=8, f=F)
                nc.vector.tensor_tensor(
                    p8[:], gv,
                    w8[:].rearrange("p t a z -> p t (a z)")
                        .unsqueeze(-1).broadcast_to([P, chunk, 8, F]),
                    _OP.mult)
                oc = wk.tile([P, chunk, F], _F32, tag="oc")
                nc.vector.tensor_reduce(
                    oc[:], p8[:].transpose([0, 1, 3, 2]),
                    axis=mybir.AxisListType.X, op=_OP.add)
                nc.sync.dma_start(out=out[:, sl, :], in_=oc[:])
    nc.compile()
    return nc


def _build_r64(table, x0):
    """R64 for x-slabs [x0, x0+16): [16*128*128, 16] f32.

    R64[(i,j,k)] = concat over (dx,dy) of table[x0+i+dx, j+dy, k:k+2, :]
    with edge clamping (out-of-range corners are never gathered).
    """
    T = np.ascontiguousarray(table, dtype=np.float32)
    xi = np.minimum(x0 + np.arange(16), RES - 1)
    out = np.empty((16, RES, RES, 4, 2, F), np.float32)
    k0 = np.arange(RES)
    k1 = np.minimum(k0 + 1, RES - 1)
    for dx in (0, 1):
        xs = np.minimum(xi + dx, RES - 1)
        for dy in (0, 1):
            ys = np.minimum(np.arange(RES) + dy, RES - 1)
            A = T[xs][:, ys]                       # [16, 128, 128, F]
            out[:, :, :, dx * 2 + dy, 0, :] = A[:, :, k0, :]
            out[:, :, :, dx * 2 + dy, 1, :] = A[:, :, k1, :]
    return out.reshape(ROWS, 16)


def _shard(c0):
    xs = c0 * np.float32(RES - 1)
    i0x_host = np.clip(np.floor(xs).astype(np.int64), 0, RES - 2)
    buckets = (i0x_host >> 4).astype(np.int64)
    perm = np.argsort(buckets, kind="stable")
    counts = np.bincount(buckets, minlength=NCORES)
    starts = np.concatenate([[0], np.cumsum(counts)[:-1]])
    maxc = int(counts.max())
    n_cols = max(1, -(-maxc // (P * CH))) * CH
    return perm, counts, starts, n_cols


def prepare_in_maps(c0, c1, c2, table):
    c0 = np.asarray(c0, np.float32)
    c1 = np.asarray(c1, np.float32)
    c2 = np.asarray(c2, np.float32)
    table = np.asarray(table, np.float32)
    perm, counts, starts, n_cols = _shard(c0)
    in_maps = []
    for c in range(NCORES):
        idx_c = perm[starts[c]:starts[c] + counts[c]]
        npad = P * n_cols
        # pad coords must land inside this core's x-slab bucket
        a0 = np.full(npad, np.float32((16 * c + 8.5) / (RES - 1)), np.float32)
        a1 = np.full(npad, np.float32(0.5), np.float32)
        a2 = np.full(npad, np.float32(0.5), np.float32)
        a0[:counts[c]] = c0[idx_c]
        a1[:counts[c]] = c1[idx_c]
        a2[:counts[c]] = c2[idx_c]
        in_maps.append({
            "c0": a0.reshape(P, n_cols),
            "c1": a1.reshape(P, n_cols),
            "c2": a2.reshape(P, n_cols),
            "R": _build_r64(table, 16 * c),
            "biasm": np.full((P, 1), -np.float32(16 * c * 16384), np.float32),
        })
    return in_maps, n_cols


def kernel(c0, c1, c2, table):
    c0 = np.asarray(c0, np.float32)
    N = c0.shape[0]
    perm, counts, starts, n_cols = _shard(c0)
    in_maps, _ = prepare_in_maps(c0, c1, c2, table)
    nc = build_core_kernel(n_cols)
    res = run_bass_kernel_spmd(nc, in_maps, core_ids=list(range(NCORES)))
    out_full = np.empty((N, F), np.float32)
    for c in range(NCORES):
        idx_c = perm[starts[c]:starts[c] + counts[c]]
        oc = np.asarray(res.results[c]["out"]).reshape(P * n_cols, F)
        out_full[idx_c] = oc[:counts[c]]
    return out_full


# revision 6
# speedup vs baseline: 3.9993x; 1.0301x over previous
"""Trilinear interpolation (trilerp) kernel for trn2, 8 NeuronCores.

Strategy:
 - Host shards the 4M points across 8 cores by x-slab bucket (i0x >> 4), a
   locality-aware data-parallel sharding of the point dimension.
 - Host replicates the (small) grid table to every core in a rearranged
   "cell-corner" layout R64: for each cell (i_rel, j, k) a contiguous 64B
   block holding all 8 corner values x 2 features, so the device gathers
   exactly one 64B block per point.
 - Device (per core): computes grid coords, fractional weights and flat cell
   indices on ACT/DVE, gathers the 64B blocks via GPSIMD indirect DMA
   (128 points per instruction, one offset per partition), and blends with
   the trilinear weights on DVE.
 - Host inverse-permutes the per-core outputs back to the original order.

All trilerp arithmetic (indices, weights, gather, blend) happens on device;
the host only permutes/lays out data.
"""
import sys
sys.path.insert(0, '/opt/trn_rl_repo')
import numpy as np

import concourse.bass as bass
import concourse.mybir as mybir
from concourse import bacc
from concourse.tile import TileContext
from concourse.bass_utils import run_bass_kernel_spmd

RES = 128
F = 2
NCORES = 8
P = 128          # partitions
CH = 128         # columns per chunk (=> 128 gathers of 128 points per chunk)
ROWS = 16 * RES * RES   # R64 rows per core (16 x-slabs x 128 x 128 cells)

_F32 = mybir.dt.float32
_I32 = mybir.dt.int32
_AF = mybir.ActivationFunctionType
_OP = mybir.AluOpType


def build_core_kernel(n_cols, rows=ROWS, chunk=CH):
    """One-core bass program: [128, n_cols] points, gather table R [rows, 16]."""
    assert n_cols % chunk == 0
    nc = bacc.Bacc("TRN2", target_bir_lowering=False, debug=False,
                   num_devices=NCORES, num_swdge_queues=4)
    c0 = nc.dram_tensor("c0", [P, n_cols], _F32, kind="ExternalInput")
    c1 = nc.dram_tensor("c1", [P, n_cols], _F32, kind="ExternalInput")
    c2 = nc.dram_tensor("c2", [P, n_cols], _F32, kind="ExternalInput")
    R = nc.dram_tensor("R", [rows, 16], _F32, kind="ExternalInput")
    biasm = nc.dram_tensor("biasm", [P, 1], _F32, kind="ExternalInput")
    out = nc.dram_tensor("out", [P, n_cols, F], _F32, kind="ExternalOutput")

    nchunks = n_cols // chunk
    with TileContext(nc) as tc:
        with tc.tile_pool(name="io", bufs=1) as io, \
             tc.tile_pool(name="wk", bufs=2) as wk, \
             tc.tile_pool(name="gp", bufs=2) as gp:
            bias_sb = io.tile([P, 1], _F32)
            nc.sync.dma_start(out=bias_sb[:], in_=biasm[:])
            for q in range(nchunks):
                sl = slice(q * chunk, (q + 1) * chunk)
                t0 = wk.tile([P, chunk], _F32, tag="t0")
                t1 = wk.tile([P, chunk], _F32, tag="t1")
                t2 = wk.tile([P, chunk], _F32, tag="t2")
                nc.sync.dma_start(out=t0[:], in_=c0[:, sl])
                nc.sync.dma_start(out=t1[:], in_=c1[:, sl])
                nc.sync.dma_start(out=t2[:], in_=c2[:, sl])

                # xs = 127*c  (ACT, Copy w/ scale; float32, same rounding as host)
                xs0 = wk.tile([P, chunk], _F32, tag="xs0")
                xs1 = wk.tile([P, chunk], _F32, tag="xs1")
                xs2 = wk.tile([P, chunk], _F32, tag="xs2")
                nc.scalar.mul(xs0[:], t0[:], float(RES - 1))
                nc.scalar.mul(xs1[:], t1[:], float(RES - 1))
                nc.scalar.mul(xs2[:], t2[:], float(RES - 1))

                # floor via round-to-nearest (+2^23 - 2^23) then correct:
                #   t = rn(xs); gt = (t > xs); i0 = t - gt; fr = xs - i0
                # exact for 0 <= xs < 2^22.
                C23 = float(1 << 23)
                wz = wk.tile([P, chunk, 2], _F32, tag="wz")
                fr0 = wk.tile([P, chunk], _F32, tag="fr0")
                fr1 = wk.tile([P, chunk], _F32, tag="fr1")
                i0x = wk.tile([P, chunk], _F32, tag="i0x")
                i0y = wk.tile([P, chunk], _F32, tag="i0y")
                i0z = wk.tile([P, chunk], _F32, tag="i0z")
                tt = wk.tile([P, chunk], _F32, tag="tt")
                gt = wk.tile([P, chunk], _F32, tag="gt")
                for xs_d, i0_d, fr_ap in (
                    (xs0, i0x, fr0[:]),
                    (xs1, i0y, fr1[:]),
                    (xs2, i0z, wz[:, :, 1]),
                ):
                    nc.vector.tensor_scalar(tt[:], xs_d[:], C23, C23,
                                            _OP.add, _OP.subtract)
                    nc.vector.tensor_tensor(gt[:], tt[:], xs_d[:], _OP.is_gt)
                    nc.vector.tensor_tensor(i0_d[:], tt[:], gt[:], _OP.subtract)
                    nc.vector.tensor_tensor(fr_ap, xs_d[:], i0_d[:], _OP.subtract)

                # m = 16384*i0x + 128*i0y + i0z + bias (bias = -16c*16384)
                ax = wk.tile([P, chunk], _F32, tag="ax")
                by = wk.tile([P, chunk], _F32, tag="by")
                nc.scalar.activation(ax[:], i0x[:], _AF.Identity,
                                     bias=bias_sb[:, 0:1], scale=float(16384))
                nc.scalar.mul(by[:], i0y[:], float(128))
                mf = wk.tile([P, chunk], _F32, tag="mf")
                nc.vector.tensor_tensor(mf[:], ax[:], by[:], _OP.add)
                nc.vector.tensor_tensor(mf[:], mf[:], i0z[:], _OP.add)
                mi = wk.tile([P, chunk], _I32, tag="mi")
                nc.vector.tensor_copy(out=mi[:], in_=mf[:])

                # weights
                omx = wk.tile([P, chunk], _F32, tag="omx")
                omy = wk.tile([P, chunk], _F32, tag="omy")
                nc.scalar.activation(omx[:], fr0[:], _AF.Identity, bias=1.0, scale=-1.0)
                nc.scalar.activation(omy[:], fr1[:], _AF.Identity, bias=1.0, scale=-1.0)
                nc.scalar.activation(wz[:, :, 0], wz[:, :, 1], _AF.Identity,
                                     bias=1.0, scale=-1.0)
                wxy = wk.tile([P, chunk, 4], _F32, tag="wxy")
                nc.vector.tensor_tensor(wxy[:, :, 0], omx[:], omy[:], _OP.mult)
                nc.vector.tensor_tensor(wxy[:, :, 1], omx[:], fr1[:], _OP.mult)
                nc.vector.tensor_tensor(wxy[:, :, 2], fr0[:], omy[:], _OP.mult)
                nc.vector.tensor_tensor(wxy[:, :, 3], fr0[:], fr1[:], _OP.mult)
                # w8[p, t, c4, z] = wxy[p, t, c4] * wz[p, t, z]
                w8 = wk.tile([P, chunk, 4, 2], _F32, tag="w8")
                nc.vector.tensor_tensor(
                    w8[:],
                    wxy[:].unsqueeze(-1).broadcast_to([P, chunk, 4, 2]),
                    wz[:].unsqueeze(2).broadcast_to([P, chunk, 4, 2]),
                    _OP.mult)

                # gather: one 64B block per point, 128 points per instruction
                g = gp.tile([P, chunk, 16], _F32, tag="g")
                for t in range(chunk):
                    inst = nc.gpsimd.indirect_dma_start(
                        out=g[:, t, :], out_offset=None, in_=R[:],
                        in_offset=bass.IndirectOffsetOnAxis(
                            ap=mi[:, t:t + 1], axis=0))
                    qi = t % 4
                    if qi:
                        inst.ins.queue = f"qPoolDynamic{qi}"
                    inst.ins.single_packet = True


                # blend: P8 = G * W8 (broadcast over f), then sum over (c4, z)
                p8 = gp.tile([P, chunk, 8, F], _F32, tag="p8")
                gv = g[:].rearrange("p t (a f) -> p t a f", a=8, f=F)
                nc.vector.tensor_tensor(
                    p8[:], gv,
                    w8[:].rearrange("p t a z -> p t (a z)")
                        .unsqueeze(-1).broadcast_to([P, chunk, 8, F]),
                    _OP.mult)
                oc = wk.tile([P, chunk, F], _F32, tag="oc")
                nc.vector.tensor_reduce(
                    oc[:], p8[:].transpose([0, 1, 3, 2]),
                    axis=mybir.AxisListType.X, op=_OP.add)
                nc.sync.dma_start(out=out[:, sl, :], in_=oc[:])
    nc.compile()
    return nc


def _build_r64(table, x0):
    """R64 for x-slabs [x0, x0+16): [16*128*128, 16] f32.

    R64[(i,j,k)] = concat over (dx,dy) of table[x0+i+dx, j+dy, k:k+2, :]
    with edge clamping (out-of-range corners are never gathered).
    """
    T = np.ascontiguousarray(table, dtype=np.float32)
    xi = np.minimum(x0 + np.arange(16), RES - 1)
    out = np.empty((16, RES, RES, 4, 2, F), np.float32)
    k0 = np.arange(RES)
    k1 = np.minimum(k0 + 1, RES - 1)
    for dx in (0, 1):
        xs = np.minimum(xi + dx, RES - 1)
        for dy in (0, 1):
            ys = np.minimum(np.arange(RES) + dy, RES - 1)
            A = T[xs][:, ys]                       # [16, 128, 128, F]
            out[:, :, :, dx * 2 + dy, 0, :] = A[:, :, k0, :]
            out[:, :, :, dx * 2 + dy, 1, :] = A[:, :, k1, :]
    return out.reshape(ROWS, 16)


def _shard(c0):
    xs = c0 * np.float32(RES - 1)
    i0x_host = np.clip(np.floor(xs).astype(np.int64), 0, RES - 2)
    buckets = (i0x_host >> 4).astype(np.int64)
    perm = np.argsort(buckets, kind="stable")
    counts = np.bincount(buckets, minlength=NCORES)
    starts = np.concatenate([[0], np.cumsum(counts)[:-1]])
    maxc = int(counts.max())
    n_cols = max(1, -(-maxc // (P * CH))) * CH
    return perm, counts, starts, n_cols


def prepare_in_maps(c0, c1, c2, table):
    c0 = np.asarray(c0, np.float32)
    c1 = np.asarray(c1, np.float32)
    c2 = np.asarray(c2, np.float32)
    table = np.asarray(table, np.float32)
    perm, counts, starts, n_cols = _shard(c0)
    in_maps = []
    for c in range(NCORES):
        idx_c = perm[starts[c]:starts[c] + counts[c]]
        npad = P * n_cols
        # pad coords must land inside this core's x-slab bucket
        a0 = np.full(npad, np.float32((16 * c + 8.5) / (RES - 1)), np.float32)
        a1 = np.full(npad, np.float32(0.5), np.float32)
        a2 = np.full(npad, np.float32(0.5), np.float32)
        a0[:counts[c]] = c0[idx_c]
        a1[:counts[c]] = c1[idx_c]
        a2[:counts[c]] = c2[idx_c]
        in_maps.append({
            "c0": a0.reshape(P, n_cols),
            "c1": a1.reshape(P, n_cols),
            "c2": a2.reshape(P, n_cols),
            "R": _build_r64(table, 16 * c),
            "biasm": np.full((P, 1), -np.float32(16 * c * 16384), np.float32),
        })
    return in_maps, n_cols


def kernel(c0, c1, c2, table):
    c0 = np.asarray(c0, np.float32)
    N = c0.shape[0]
    perm, counts, starts, n_cols = _shard(c0)
    in_maps, _ = prepare_in_maps(c0, c1, c2, table)
    nc = build_core_kernel(n_cols)
    res = run_bass_kernel_spmd(nc, in_maps, core_ids=list(range(NCORES)))
    out_full = np.empty((N, F), np.float32)
    for c in range(NCORES):
        idx_c = perm[starts[c]:starts[c] + counts[c]]
        oc = np.asarray(res.results[c]["out"]).reshape(P * n_cols, F)
        out_full[idx_c] = oc[:counts[c]]
    return out_full


# revision 12
# speedup vs baseline: 8.1977x; 2.0498x over previous
"""Trilerp kernel v2: same-cell pair dedup halves gathers for ~70% of points.

Host sorts each core's points by cell and pairs same-cell points; each pair
shares one gathered 64B block. Pairs segment: chunks of 256 point-cols whose
first 128 cols (A) drive the 128 gathers; B cols reuse the same gathered tile
in a second blend. Singles segment: original one-gather-per-point path.
"""
import sys
sys.path.insert(0, '/opt/trn_rl_repo')
import numpy as np

import concourse.bass as bass
import concourse.mybir as mybir
from concourse import bacc
from concourse.tile import TileContext
from concourse.bass_utils import run_bass_kernel_spmd

RES = 128
F = 2
NCORES = 8
P = 128
CH = 128
ROWS = 16 * RES * RES
_F32 = mybir.dt.float32
_I32 = mybir.dt.int32
_AF = mybir.ActivationFunctionType
_OP = mybir.AluOpType


def build_core_kernel(n_chunks_by_k, rows=ROWS):
    """n_chunks_by_k[k] chunks of group-size k (k*CH cols, CH gathers each)."""
    n_cols = sum(n_chunks_by_k[k] * k * CH for k in n_chunks_by_k)
    nc = bacc.Bacc("TRN2", target_bir_lowering=False, debug=False,
                   num_devices=NCORES, num_swdge_queues=4)
    c0 = nc.dram_tensor("c0", [P, n_cols], _F32, kind="ExternalInput")
    c1 = nc.dram_tensor("c1", [P, n_cols], _F32, kind="ExternalInput")
    c2 = nc.dram_tensor("c2", [P, n_cols], _F32, kind="ExternalInput")
    R = nc.dram_tensor("R", [rows, 16], _F32, kind="ExternalInput")
    biasm = nc.dram_tensor("biasm", [P, 1], _F32, kind="ExternalInput")
    out = nc.dram_tensor("out", [P, n_cols, F], _F32, kind="ExternalOutput")

    with TileContext(nc) as tc:
        with tc.tile_pool(name="io", bufs=1) as io, \
             tc.tile_pool(name="wk", bufs=2) as wk, \
             tc.tile_pool(name="gp", bufs=2) as gp:
            bias_sb = io.tile([P, 1], _F32)
            nc.sync.dma_start(out=bias_sb[:], in_=biasm[:])

            def chunk_body(col0, w, nrep):
                # w = nrep*CH point-cols; first CH cols drive the gathers,
                # the other nrep-1 col-groups reuse the gathered tile.
                paired = nrep
                sl = slice(col0, col0 + w)
                t0 = wk.tile([P, w], _F32, tag="t0")
                t1 = wk.tile([P, w], _F32, tag="t1")
                t2 = wk.tile([P, w], _F32, tag="t2")
                nc.sync.dma_start(out=t0[:], in_=c0[:, sl])
                nc.sync.dma_start(out=t1[:], in_=c1[:, sl])
                nc.sync.dma_start(out=t2[:], in_=c2[:, sl])
                xs0 = wk.tile([P, w], _F32, tag="xs0")
                xs1 = wk.tile([P, w], _F32, tag="xs1")
                xs2 = wk.tile([P, w], _F32, tag="xs2")
                nc.scalar.mul(xs0[:], t0[:], float(RES - 1))
                nc.scalar.mul(xs1[:], t1[:], float(RES - 1))
                nc.scalar.mul(xs2[:], t2[:], float(RES - 1))
                C23 = float(1 << 23)
                wz = wk.tile([P, w, 2], _F32, tag="wz")
                fr0 = wk.tile([P, w], _F32, tag="fr0")
                fr1 = wk.tile([P, w], _F32, tag="fr1")
                i0x = wk.tile([P, w], _F32, tag="i0x")
                i0y = wk.tile([P, w], _F32, tag="i0y")
                i0z = wk.tile([P, w], _F32, tag="i0z")
                tt = wk.tile([P, w], _F32, tag="tt")
                gt = wk.tile([P, w], _F32, tag="gt")
                for xs_d, i0_d, fr_ap in ((xs0, i0x, fr0[:]), (xs1, i0y, fr1[:]),
                                          (xs2, i0z, wz[:, :, 1])):
                    nc.vector.tensor_scalar(tt[:], xs_d[:], C23, C23,
                                            _OP.add, _OP.subtract)
                    nc.vector.tensor_tensor(gt[:], tt[:], xs_d[:], _OP.is_gt)
                    nc.vector.tensor_tensor(i0_d[:], tt[:], gt[:], _OP.subtract)
                    nc.vector.tensor_tensor(fr_ap, xs_d[:], i0_d[:], _OP.subtract)
                ax = wk.tile([P, w], _F32, tag="ax")
                by = wk.tile([P, w], _F32, tag="by")
                nc.scalar.activation(ax[:], i0x[:], _AF.Identity,
                                     bias=bias_sb[:, 0:1], scale=float(16384))
                nc.scalar.mul(by[:], i0y[:], float(128))
                mf = wk.tile([P, w], _F32, tag="mf")
                nc.vector.tensor_tensor(mf[:], ax[:], by[:], _OP.add)
                nc.vector.tensor_tensor(mf[:], mf[:], i0z[:], _OP.add)
                mi = wk.tile([P, CH], _I32, tag="mi")
                nc.vector.tensor_copy(out=mi[:], in_=mf[:, 0:CH])
                omx = wk.tile([P, w], _F32, tag="omx")
                omy = wk.tile([P, w], _F32, tag="omy")
                nc.scalar.activation(omx[:], fr0[:], _AF.Identity, bias=1.0, scale=-1.0)
                nc.scalar.activation(omy[:], fr1[:], _AF.Identity, bias=1.0, scale=-1.0)
                nc.scalar.activation(wz[:, :, 0], wz[:, :, 1], _AF.Identity,
                                     bias=1.0, scale=-1.0)
                wxy = wk.tile([P, w, 4], _F32, tag="wxy")
                nc.vector.tensor_tensor(wxy[:, :, 0], omx[:], omy[:], _OP.mult)
                nc.vector.tensor_tensor(wxy[:, :, 1], omx[:], fr1[:], _OP.mult)
                nc.vector.tensor_tensor(wxy[:, :, 2], fr0[:], omy[:], _OP.mult)
                nc.vector.tensor_tensor(wxy[:, :, 3], fr0[:], fr1[:], _OP.mult)
                w8 = wk.tile([P, w, 4, 2], _F32, tag="w8")
                nc.vector.tensor_tensor(
                    w8[:], wxy[:].unsqueeze(-1).broadcast_to([P, w, 4, 2]),
                    wz[:].unsqueeze(2).broadcast_to([P, w, 4, 2]), _OP.mult)

                g = gp.tile([P, CH, 16], _F32, tag="g")
                for t in range(CH):
                    inst = nc.gpsimd.indirect_dma_start(
                        out=g[:, t, :], out_offset=None, in_=R[:],
                        in_offset=bass.IndirectOffsetOnAxis(
                            ap=mi[:, t:t + 1], axis=0))
                    qi = t % 4
                    if qi:
                        inst.ins.queue = f"qPoolDynamic{qi}"
                    inst.ins.single_packet = True

                gv = g[:].rearrange("p t (a f) -> p t a f", a=8, f=F)
                w8v = w8[:].rearrange("p t a z -> p t (a z)")
                oc = wk.tile([P, w, F], _F32, tag="oc")
                for j in range(nrep):
                    p8 = gp.tile([P, CH, 8, F], _F32, tag="p8")
                    nc.vector.tensor_tensor(
                        p8[:], gv,
                        w8v[:, j * CH:(j + 1) * CH].unsqueeze(-1)
                            .broadcast_to([P, CH, 8, F]),
                        _OP.mult)
                    nc.vector.tensor_reduce(
                        oc[:, j * CH:(j + 1) * CH],
                        p8[:].transpose([0, 1, 3, 2]),
                        axis=mybir.AxisListType.X, op=_OP.add)
                nc.sync.dma_start(out=out[:, sl, :], in_=oc[:])

            col = 0
            for k in sorted(n_chunks_by_k, reverse=True):
                for _ in range(n_chunks_by_k[k]):
                    chunk_body(col, k * CH, k)
                    col += k * CH
    nc.compile()
    return nc


def _build_r64(table, x0):
    T = np.ascontiguousarray(table, dtype=np.float32)
    xi = np.minimum(x0 + np.arange(16), RES - 1)
    out = np.empty((16, RES, RES, 4, 2, F), np.float32)
    k0 = np.arange(RES)
    k1 = np.minimum(k0 + 1, RES - 1)
    for dx in (0, 1):
        xs = np.minimum(xi + dx, RES - 1)
        for dy in (0, 1):
            ys = np.minimum(np.arange(RES) + dy, RES - 1)
            A = T[xs][:, ys]
            out[:, :, :, dx * 2 + dy, 0, :] = A[:, :, k0, :]
            out[:, :, :, dx * 2 + dy, 1, :] = A[:, :, k1, :]
    return out.reshape(ROWS, 16)


def _host_layout(c0):
    """Bucket by x-slab, then per core sort by cell and pair same-cell points.

    Returns per-core (A_idx, B_idx, S_idx) original-index arrays and the
    common (n_pc, n_sc) chunk counts.
    """
    xs0 = c0 * np.float32(RES - 1)
    i0x = np.clip(np.floor(xs0).astype(np.int64), 0, RES - 2)
    buckets = i0x >> 4
    per_core = []
    for c in range(NCORES):
        idx_c = np.flatnonzero(buckets == c)
        per_core.append(idx_c)
    return per_core


def kernel(c0, c1, c2, table):
    c0 = np.asarray(c0, np.float32)
    c1 = np.asarray(c1, np.float32)
    c2 = np.asarray(c2, np.float32)
    table = np.asarray(table, np.float32)
    N = c0.shape[0]
    PTS = P * CH

    xs = [a * np.float32(RES - 1) for a in (c0, c1, c2)]
    i0 = [np.clip(np.floor(x).astype(np.int64), 0, RES - 2) for x in xs]
    buckets = i0[0] >> 4
    m_all = (i0[0] - 16 * buckets) * 16384 + i0[1] * 128 + i0[2]

    KS = (4, 3, 2, 1)
    cores = []
    for c in range(NCORES):
        idx_c = np.flatnonzero(buckets == c)
        ms = m_all[idx_c]
        order = np.argsort(ms, kind="stable")
        srt = idx_c[order]
        msr = ms[order]
        n = len(srt)
        new_run = np.ones(n, bool)
        if n > 1:
            new_run[1:] = msr[1:] != msr[:-1]
        starts = np.flatnonzero(new_run)
        runlen = np.diff(np.append(starts, n))
        rid = np.cumsum(new_run) - 1
        pos = np.arange(n) - starts[rid]
        rl = runlen[rid]
        nfull = 4 * (rl // 4)
        in_quad = pos < nfull
        k_of = np.where(in_quad, 4, rl % 4)
        j_of = np.where(in_quad, pos % 4, pos - nfull)
        lists = {k: [srt[(k_of == k) & (j_of == j)] for j in range(k)]
                 for k in KS}
        cores.append(lists)

    n_chunks_by_k = {k: max(-(-len(cores[c][k][0]) // PTS)
                            for c in range(NCORES)) for k in KS}
    n_cols = sum(n_chunks_by_k[k] * k * CH for k in KS)
    base_k = {}
    col = 0
    for k in KS:
        base_k[k] = col
        col += n_chunks_by_k[k] * k * CH

    nc = build_core_kernel(n_chunks_by_k)

    in_maps = []
    for c in range(NCORES):
        lists = cores[c]
        pad0 = np.float32((16 * c + 8.5) / (RES - 1))
        a0 = np.full((P, n_cols), pad0, np.float32)
        a1 = np.full((P, n_cols), np.float32(0.5), np.float32)
        a2 = np.full((P, n_cols), np.float32(0.5), np.float32)
        for k in KS:
            for q in range(n_chunks_by_k[k]):
                for j in range(k):
                    pts = lists[k][j][q * PTS:(q + 1) * PTS]
                    col0 = base_k[k] + q * k * CH + j * CH
                    b0 = np.full(PTS, pad0, np.float32); b0[:len(pts)] = c0[pts]
                    b1 = np.full(PTS, np.float32(0.5), np.float32); b1[:len(pts)] = c1[pts]
                    b2 = np.full(PTS, np.float32(0.5), np.float32); b2[:len(pts)] = c2[pts]
                    a0[:, col0:col0 + CH] = b0.reshape(P, CH)
                    a1[:, col0:col0 + CH] = b1.reshape(P, CH)
                    a2[:, col0:col0 + CH] = b2.reshape(P, CH)
        in_maps.append({
            "c0": a0, "c1": a1, "c2": a2,
            "R": _build_r64(table, 16 * c),
            "biasm": np.full((P, 1), -np.float32(16 * c * 16384), np.float32),
        })

    res = run_bass_kernel_spmd(nc, in_maps, core_ids=list(range(NCORES)))

    out_full = np.empty((N, F), np.float32)
    for c in range(NCORES):
        lists = cores[c]
        oc = np.asarray(res.results[c]["out"])
        for k in KS:
            for q in range(n_chunks_by_k[k]):
                for j in range(k):
                    pts = lists[k][j][q * PTS:(q + 1) * PTS]
                    col0 = base_k[k] + q * k * CH + j * CH
                    blk = oc[:, col0:col0 + CH, :].reshape(PTS, F)
                    out_full[pts] = blk[:len(pts)]
    return out_full
